# revision 22
# baseline (speedup 1.0000x reference)
"""Trainium2 Bass kernel for the MAB (multihead attention block) problem.

Full inputs in, full outputs out. Data-parallel over batch: 16 batches
across 8 NeuronCores = 2 batches/core. No collectives.

Per-core pipeline (per batch):
  1. QpT = (Q @ Wq + bq)^T bf16, KpT likewise, Vp natural fp8 (+ ones
     column per head for the softmax denominator; head stride padded to
     80 bytes for the DoubleRow LDWEIGHTS alignment rule).
  2. Per head pair: S^T = Kh @ Qh^T (row-tiled K=64 pairs sharing the PE
     array), P = exp(S^T*s) written as fp8 (no max subtraction -- scores
     are N(0, 0.35), exp is safe), O'^T[66, nq] = [Vh | 1]^T @ P
     accumulated over nk-tile PAIRS with fp8 DoubleRow matmuls (2
     contraction rows per PE cell = half the matmul count; row 64 of the
     output = softmax denominator).
  3. Transpose O'^T to natural (bf16), normalize rows by 1/denominator,
     add Qp residual.
  4. LayerNorm -> transpose -> FFN relu(X @ Wo)+X fused on DVE ->
     LayerNorm -> out. rsqrt computed as exp(-0.5*ln(var+eps)) so the
     ACT table set never leaves exp (a sqrt would force a ~2.7us
     ACT_TABLE_LOAD round-trip per switch). Affine params that are
     identically (1, 0) are folded out at build time.

Emission order is shaped for Tile's priority scheduler: a minimal
projection head (qpt/kpt chunk 0, vpa m0-1) precedes the first attention
group so the exp stream starts early; the remaining projection /
transpose chunks and the next batch's phase A are emitted between
attention groups so they fill the exp-paced PE bubbles.
"""

import math
import sys
from contextlib import ExitStack

import numpy as np

sys.path.insert(0, "/opt/trn_rl_repo")

import concourse.bass as bass
import concourse.tile as tile
from concourse import bacc
from concourse import mybir
from concourse.bass import ds, ts
from concourse.bass_utils import run_bass_kernel_spmd
from concourse.masks import make_identity

FP = mybir.dt.float32
AF = mybir.ActivationFunctionType
ALU = mybir.AluOpType
BF = mybir.dt.bfloat16
F8 = mybir.dt.float8e4
DR = mybir.MatmulPerfMode.DoubleRow

B, N, D = 16, 1024, 512
NCORES = 8
BL = B // NCORES  # batches per core
H, HD = 8, 64
SCALE = 1.0 / math.sqrt(D)
EPS = 1e-5
P = 128
DT = D // P  # 4 dv chunks
NT = N // P  # 8 nq/nk tiles
HA = HD + 1  # head dim + denominator column
HP8 = 80     # padded per-head stride in the fp8 V tile (16B-aligned)


def _bcast_ap(ap):
    """Broadcast a [D]-shaped DRAM AP across all 128 partitions."""
    return bass.AP(tensor=ap.tensor, offset=ap.offset, ap=[[0, P]] + list(ap.ap))


def _build_program(triv0, triv1, trivbo, trivqk):
    nc = bacc.Bacc(None, target_bir_lowering=False)
    dr = {}
    for name, shape in [
        ("QT", [BL, D, N]),
        ("KT", [BL, D, N]),
        ("Wq", [D, D]),
        ("Wk", [D, D]),
        ("Wv", [D, D]),
        ("Wo", [D, D]),
        ("bq2", [P, DT]),
        ("bq", [D]),
        ("bk2", [P, DT]),
        ("bv", [D]),
        ("bo", [D]),
        ("g0", [D]),
        ("b0", [D]),
        ("g1", [D]),
        ("b1", [D]),
    ]:
        if name in ("KT", "Wk", "Wv"):
            dt = F8
        elif name in ("QT", "Wq", "Wo"):
            dt = BF
        else:
            dt = FP
        dr[name] = nc.declare_dram_parameter(name, shape, dt, isOutput=False)
    out_O = nc.declare_dram_parameter("O", [BL, N, D], FP, isOutput=True)
    import os
    DBG = bool(os.environ.get("KDBG"))
    if DBG:
        dbg_qp = nc.declare_dram_parameter("DQP", [BL, NT, P, D], BF, isOutput=True)
        dbg_oasm = nc.declare_dram_parameter("DOASM", [BL, NT, P, D], BF, isOutput=True)
        dbg_ln1 = nc.declare_dram_parameter("DLN1", [BL, NT, P, D], BF, isOutput=True)

    qt_src = dr["QT"][:].rearrange("b (c p) n -> b p c n", p=P)
    kt_src = dr["KT"][:].rearrange("b (c p) n -> b p c n", p=P)

    with tile.TileContext(nc) as tc, ExitStack() as ctx:
        singles = ctx.enter_context(tc.tile_pool(name="singles", bufs=1))
        work = ctx.enter_context(tc.tile_pool(name="work", bufs=2))
        pch = ctx.enter_context(tc.tile_pool(name="pch", bufs=2))
        lnt = ctx.enter_context(tc.tile_pool(name="lnt", bufs=2))
        ost = ctx.enter_context(tc.tile_pool(name="ost", bufs=2))
        otile = ctx.enter_context(tc.tile_pool(name="otile", bufs=2))
        o2p = ctx.enter_context(tc.tile_pool(name="o2p", bufs=5))
        sml = ctx.enter_context(tc.tile_pool(name="sml", bufs=8))
        # PSUM budget: flow 2x2 banks + opv 2x1 banks + acc 2x1 banks = 8
        ps_acc = ctx.enter_context(tc.tile_pool(name="ps_acc", bufs=2, space="PSUM"))
        ps_pv = ctx.enter_context(tc.tile_pool(name="ps_pv", bufs=1, space="PSUM"))
        ps_flow = ctx.enter_context(tc.tile_pool(name="ps_flow", bufs=2, space="PSUM"))

        # ---- statics
        wsb = {}
        for wname in ("Wq", "Wk", "Wv", "Wo"):
            wdt = F8 if wname in ("Wk", "Wv") else BF
            wsb[wname] = singles.tile([P, DT, D], wdt, tag=wname, name=wname)
        nc.sync.dma_start(
            out=wsb["Wq"], in_=dr["Wq"][:].rearrange("(c p) d -> p c d", p=P)
        )
        bq_sb = singles.tile([P, DT], FP, tag="bq2")
        nc.sync.dma_start(out=bq_sb, in_=dr["bq2"][:])
        bk_sb = singles.tile([P, DT], FP, tag="bk2")
        bc = {}
        for bname in ("bq", "bv", "bo", "g0", "b0", "g1", "b1"):
            t = singles.tile([P, D], FP, tag=bname)
            nc.gpsimd.dma_start(out=t, in_=_bcast_ap(dr[bname][:]))
            bc[bname] = t
        ident = singles.tile([P, P], FP, tag="ident")
        make_identity(nc, ident)
        ident_b = singles.tile([P, P], BF, tag="identb")
        nc.vector.tensor_copy(ident_b, ident)
        eps_sb = singles.tile([P, 1], FP, tag="eps")
        nc.vector.memset(eps_sb, EPS)

        state = {}

        def rsqrt_dve(out4, var_ap):
            """out4[P,4] = 1/sqrt(var+eps) via bit-trick seed + 2 Newton
            iterations, all on DVE ALU ops (no ACT table traffic)."""
            I32 = mybir.dt.int32
            v4 = sml.tile([P, 4], FP, tag="v4", name="v4")
            nc.vector.tensor_scalar_add(v4, var_ap, EPS)
            yi = out4.bitcast(I32)
            nc.vector.tensor_scalar(
                out=yi, in0=v4.bitcast(I32), scalar1=1, scalar2=None,
                op0=ALU.arith_shift_right,
            )
            nc.vector.tensor_scalar(
                out=yi, in0=yi, scalar1=0x5F3759DF, scalar2=-1,
                op0=ALU.subtract, op1=ALU.mult,
            )
            h4 = sml.tile([P, 4], FP, tag="h4", name="h4")
            for _ in range(2):
                nc.vector.tensor_tensor(h4, out4, out4, ALU.mult)
                nc.vector.tensor_tensor(h4, h4, v4, ALU.mult)
                nc.vector.tensor_scalar(
                    out=h4, in0=h4, scalar1=-0.5, scalar2=1.5,
                    op0=ALU.mult, op1=ALU.add,
                )
                nc.vector.tensor_tensor(out4, out4, h4, ALU.mult)

        def proj_chunk(b, t):
            """One dv-chunk of the QpT (bf16) and KpT (fp8 DoubleRow)
            projections."""
            st = state[b]
            for hf in range(2):
                ps = ps_acc.tile([P, 512], FP, tag="acc", name="projps")
                for c in range(DT):
                    nc.tensor.matmul(
                        ps,
                        (wsb["Wq"][:, c, ts(t, P)]),
                        (st["qt"][:, c, ds(hf * 512, 512)]),
                        start=(c == 0),
                        stop=(c == DT - 1),
                    )
                if trivqk and b == 0:
                    nc.scalar.copy(st["qpt"][:, t, ds(hf * 512, 512)], ps)
                else:
                    nc.vector.tensor_scalar_add(
                        st["qpt"][:, t, ds(hf * 512, 512)], ps, bq_sb[:, t : t + 1]
                    )
            for hf in range(2):
                ps = ps_acc.tile([P, 512], FP, tag="acc", name="projps")
                for ci in range(DT // 2):
                    nc.tensor.matmul(
                        ps,
                        wsb["Wk"][:, ds(2 * ci, 2), ts(t, P)],
                        st["kt"][:, ds(2 * ci, 2), ds(hf * 512, 512)],
                        start=(ci == 0),
                        stop=(ci == DT // 2 - 1),
                        perf_mode=DR,
                    )
                if trivqk and b == 0:
                    nc.scalar.copy(st["kpt"][:, t, ds(hf * 512, 512)], ps)
                else:
                    nc.vector.tensor_scalar_add(
                        st["kpt"][:, t, ds(hf * 512, 512)], ps, bk_sb[:, t : t + 1]
                    )

        def vpa_chunk(b, ms):
            """Vp natural fp8 rows for nk-tiles ms."""
            st = state[b]
            kt = st["kt"]
            va = st["vpa"][:, :, :].rearrange("p m (h s) -> p m h s", s=HP8)
            for m in ms:
                ps = ps_acc.tile([P, 512], FP, tag="acc", name="vps")
                for ci in range(DT // 2):
                    nc.tensor.matmul(
                        ps,
                        kt[:, ds(2 * ci, 2), ts(m, P)],
                        wsb["Wv"][:, ds(2 * ci, 2), :],
                        start=(ci == 0),
                        stop=(ci == DT // 2 - 1),
                        perf_mode=DR,
                    )
                nc.vector.scalar_tensor_tensor(
                    out=va[:, m, :, 0:HD],
                    in0=ps[:, :].rearrange("p (h s) -> p h s", s=HD),
                    scalar=0.0,
                    in1=bc["bv"][:, :].rearrange("p (h s) -> p h s", s=HD),
                    op0=ALU.bypass,
                    op1=ALU.add,
                )

        def qp_mm(b, ms):
            """Natural-layout Qp rows (attention residual) as a direct
            matmul: qp[q, :] = Q[q, :] @ Wq + bq, with the QT input tile
            as the stationary operand. No transposes needed."""
            st = state[b]
            qt, qp = st["qt"], st["qp"]
            for m in ms:
                ps = ps_acc.tile([P, 512], FP, tag="acc", name="qpps")
                for c in range(DT):
                    nc.tensor.matmul(
                        ps,
                        (qt[:, c, ts(m, P)]),
                        (wsb["Wq"][:, c, :]),
                        start=(c == 0),
                        stop=(c == DT - 1),
                    )
                nc.vector.scalar_tensor_tensor(
                    out=qp[:, m, :],
                    in0=ps,
                    scalar=0.0,
                    in1=bc["bq"],
                    op0=ALU.bypass,
                    op1=ALU.add,
                )

        def phase_a_dma(b):
            st = {}
            state[b] = st
            qt = work.tile([P, DT, N], BF, tag="qt")
            kt = work.tile([P, DT, N], F8, tag="kt")
            for c in range(DT):
                nc.scalar.dma_start(out=qt[:, c, :], in_=qt_src[b, :, c, :])
                nc.sync.dma_start(out=kt[:, c, :], in_=kt_src[b, :, c, :])
            if b == 0:
                for wname in ("Wk", "Wo", "Wv"):
                    nc.sync.dma_start(
                        out=wsb[wname],
                        in_=dr[wname][:].rearrange("(c p) d -> p c d", p=P),
                    )
                nc.sync.dma_start(out=bk_sb, in_=dr["bk2"][:])
            st["qt"], st["kt"] = qt, kt

        def phase_a_head(b):
            st = state[b]
            st["qpt"] = work.tile([P, DT, N], BF, tag="qpt", name="qpt")
            st["kpt"] = work.tile([P, DT, N], BF, tag="kpt", name="kpt")
            vpa = work.tile([P, NT, H * HP8], F8, tag="vpa", name="vpa")
            st["vpa"] = vpa
            va = vpa[:, :, :].rearrange("p m (h s) -> p m h s", s=HP8)
            nc.vector.memset(va[:, :, :, HD : HD + 1], 1.0)
            nc.vector.memset(va[:, :, :, HD + 1 : HD + 2], 0.0)
            st["qp"] = work.tile([P, NT, D], BF, tag="qp", name="qp")
            st["oasm"] = work.tile([P, NT, D], BF, tag="oasm", name="oasm")
            st["ln1"] = work.tile([P, NT, D], BF, tag="ln1", name="ln1")
            proj_chunk(b, 0)
            vpa_chunk(b, (0, 1, 2, 3, 4, 5, 6, 7))
            qp_mm(b, (0, 1, 2, 3))
            st["rest"] = [
                lambda b=b: (proj_chunk(b, 1), qp_mm(b, (4, 5))),
                lambda b=b: (proj_chunk(b, 2), qp_mm(b, (6, 7))),
                lambda b=b: (proj_chunk(b, 3),),
            ]

        def drain_group(b, hp, hf, o_pair):
            """PSUM O'^T -> natural bf16, normalize, add Qp residual."""
            st = state[b]
            qp, oasm = st["qp"], st["oasm"]
            o_sb = ost.tile([HA, N], BF, tag="ost", name="osb")
            nc.vector.tensor_copy(o_sb[:, 0:512], o_pair[0:HA, 0:512])
            nc.vector.tensor_copy(o_sb[:, 512:1024], o_pair[0:HA, 512:1024])
            for j in range(2):
                h = 2 * hp + j
                # stride 66 keeps each bf16 PSUM write 4-byte aligned
                t_ps = ps_acc.tile([P, 4 * 66], BF, tag="acc", name="otr")
                for qq in range(4):
                    nc.tensor.transpose(
                        t_ps[:, ds(qq * 66, HA)],
                        o_sb[:, ds(j * 512 + qq * P, P)],
                        ident_b[0:HA, 0:HA],
                    )
                r4 = sml.tile([P, 4], FP, tag="r", name="r4")
                den = bass.AP(
                    tensor=t_ps.tensor,
                    offset=t_ps.offset + HD,
                    ap=[list(t_ps.ap[0]), [66, 4]],
                )
                nc.vector.reciprocal(r4, den)
                for qq in range(4):
                    q = hf * 4 + qq
                    nc.vector.scalar_tensor_tensor(
                        out=oasm[:, q, ds(h * HD, HD)],
                        in0=t_ps[:, ds(qq * 66, HD)],
                        scalar=r4[:, qq : qq + 1],
                        in1=qp[:, q, ds(h * HD, HD)],
                        op0=ALU.mult,
                        op1=ALU.add,
                    )

        def attn_group(b, hf, hp, mid=None):
            st = state[b]
            qpt, kpt, vpa = st["qpt"], st["kpt"], st["vpa"]
            qslice = ds(hf * 512, 512)
            o_pair = ps_pv.tile([HA + 1, N], FP, tag="opv", name="opv")

            p8 = pch.tile([P, NT, N], F8, tag="p8")

            def emit_pv(m2):
                for j in range(2):
                    h = 2 * hp + j
                    nc.tensor.matmul(
                        o_pair[:, ds(j * 512, 512)],
                        vpa[:, ds(2 * m2, 2), ds(h * HP8, HA + 1)],
                        p8[:, ds(2 * m2, 2), ds(j * 512, 512)],
                        start=(m2 == 0),
                        stop=(m2 == NT // 2 - 1),
                        perf_mode=DR,
                    )

            pend = []
            for m in range(NT):
                s_pair = ps_flow.tile([P, N], FP, tag="flow", name="spair")
                for j in range(2):
                    lo = j * 64
                    nc.tensor.matmul(
                        s_pair[:, ds(j * 512, 512)],
                        (kpt[lo : lo + 64, hp, ts(m, P)]),
                        (qpt[lo : lo + 64, hp, qslice]),
                        start=True,
                        stop=True,
                    )
                # PV pairs and the previous group's drain are emitted after
                # this m's scores but before its exp: the scores (and thus
                # the exp stream) win scheduler priority over them.
                if m == 1 and pending_drain[0] is not None:
                    pending_drain[0]()
                    pending_drain[0] = None
                if len(pend) > 1:
                    emit_pv(pend.pop(0))
                nc.scalar.activation(p8[:, m, :], s_pair, AF.Exp, scale=SCALE)
                if mid and m in mid:
                    mid[m]()
                if m % 2 == 1:
                    pend.append(m // 2)
            for m2 in pend:
                emit_pv(m2)
            pending_drain[0] = lambda b=b, hp=hp, hf=hf, o=o_pair: drain_group(
                b, hp, hf, o
            )

        def phase_c_half(b, hf):
            """LN1 + FFN + LN2 + store for q-tiles 4*hf .. 4*hf+3."""
            st = state[b]
            oasm, ln1 = st["oasm"], st["ln1"]
            mva = sml.tile([P, 4, 2], FP, tag="mva", name="mva")
            for qq in range(4):
                st_ = sml.tile([P, 6], FP, tag="bn", name="st")
                nc.vector.bn_stats(st_, oasm[:, hf * 4 + qq, :])
                nc.vector.bn_aggr(mva[:, qq, :], st_)
            rsa = sml.tile([P, 4], FP, tag="rsa", name="rsa")
            rsqrt_dve(rsa, mva[:, :, 1])
            for qq in range(4):
                q = hf * 4 + qq
                lq = ln1[:, q, :]
                nc.vector.tensor_scalar(
                    out=lq,
                    in0=oasm[:, q, :],
                    scalar1=mva[:, qq, 0:1],
                    scalar2=rsa[:, qq : qq + 1],
                    op0=ALU.subtract,
                    op1=ALU.mult,
                )
                if not triv0:
                    nc.vector.tensor_tensor(lq, lq, bc["g0"], ALU.mult)
                    nc.vector.tensor_tensor(lq, lq, bc["b0"], ALU.add)

            mvb = sml.tile([P, 4, 2], FP, tag="mvb", name="mvb")
            o2s = []
            for qq in range(4):
                q = hf * 4 + qq
                lq = ln1[:, q, :]
                tp = ps_acc.tile([P, 512], BF, tag="acc", name="lntr")
                for c in range(DT):
                    nc.tensor.transpose(tp[:, ts(c, P)], lq[:, ts(c, P)], ident_b)
                l_t = lnt.tile([P, DT, P], BF, tag="lnt", name="lt")
                nc.vector.tensor_copy(l_t, tp.rearrange("p (c n) -> p c n", n=P))

                f_ps = ps_acc.tile([P, 512], FP, tag="acc", name="ffps")
                for c in range(DT):
                    nc.tensor.matmul(
                        f_ps,
                        (l_t[:, c, :]),
                        (wsb["Wo"][:, c, :]),
                        start=(c == 0),
                        stop=(c == DT - 1),
                    )
                o2 = o2p.tile([P, D], BF, tag="o2", name="o2")
                o2s.append(o2)
                if trivbo:
                    # o2 = relu(f_ps) + ln1 in one DVE op
                    nc.vector.scalar_tensor_tensor(
                        out=o2,
                        in0=f_ps,
                        scalar=0.0,
                        in1=lq,
                        op0=ALU.max,
                        op1=ALU.add,
                    )
                else:
                    nc.vector.tensor_tensor(o2, f_ps, bc["bo"], ALU.add)
                    nc.vector.tensor_scalar_max(o2, o2, 0.0)
                    nc.vector.tensor_tensor(o2, o2, lq, ALU.add)
                st2 = sml.tile([P, 6], FP, tag="bn", name="st2")
                nc.vector.bn_stats(st2, o2)
                nc.vector.bn_aggr(mvb[:, qq, :], st2)
            rsb = sml.tile([P, 4], FP, tag="rsb", name="rsb")
            rsqrt_dve(rsb, mvb[:, :, 1])
            for qq in range(4):
                q = hf * 4 + qq
                z2 = otile.tile([P, D], FP, tag="z", name="z2")
                nc.vector.tensor_scalar(
                    out=z2,
                    in0=o2s[qq],
                    scalar1=mvb[:, qq, 0:1],
                    scalar2=rsb[:, qq : qq + 1],
                    op0=ALU.subtract,
                    op1=ALU.mult,
                )
                if not triv1:
                    nc.vector.tensor_tensor(z2, z2, bc["g1"], ALU.mult)
                    nc.vector.tensor_tensor(z2, z2, bc["b1"], ALU.add)
                nc.sync.dma_start(out=out_O[b, ts(q, P), :], in_=z2)

        # ---- emission order (shapes scheduler priorities) ----
        pending_drain = [None]

        def flush_drain():
            if pending_drain[0] is not None:
                pending_drain[0]()
                pending_drain[0] = None

        phase_a_dma(0)
        phase_a_head(0)
        while state[0]["rest"]:
            state[0]["rest"].pop(0)()
        for b in range(BL):
            for hp in range(4):
                attn_group(b, 0, hp)
            if b + 1 < BL:
                # input DMAs submitted early: the sync DGE queue is FIFO,
                # so they must precede the c-half work on that queue
                phase_a_dma(b + 1)
            if b + 1 == BL:
                # last batch: overlap the first c-half with hf=1 attention
                flush_drain()
                phase_c_half(b, 0)
            for hp in range(4):
                attn_group(b, 1, hp)
            flush_drain()
            # emitted after all of this batch's groups: lower priority,
            # so they fill the exp-paced PE bubbles instead of delaying
            # groups. Next batch's projections go FIRST so its attention
            # restarts promptly; this batch's c-halves then overlap it.
            if b + 1 < BL:
                phase_a_head(b + 1)
                while state[b + 1]["rest"]:
                    state[b + 1]["rest"].pop(0)()
                phase_c_half(b, 0)
            phase_c_half(b, 1)
            if DBG:
                for m in range(NT):
                    nc.sync.dma_start(out=dbg_qp[b, m], in_=state[b]["qp"][:, m, :])
                    nc.sync.dma_start(out=dbg_oasm[b, m], in_=state[b]["oasm"][:, m, :])
                    nc.sync.dma_start(out=dbg_ln1[b, m], in_=state[b]["ln1"][:, m, :])

    nc.compile()
    return nc


_NC = {}


def _get_nc(triv0, triv1, trivbo, trivqk=True):
    key = (triv0, triv1, trivbo, trivqk)
    if key not in _NC:
        _NC[key] = _build_program(*key)
    return _NC[key]


def _prep_in_maps(inputs):
    import ml_dtypes

    f32 = lambda x: np.ascontiguousarray(np.asarray(x), dtype=np.float32)
    bf = lambda x: np.ascontiguousarray(np.asarray(x, dtype=np.float32).astype(ml_dtypes.bfloat16))
    f8 = lambda x: np.ascontiguousarray(np.asarray(x, dtype=np.float32).astype(ml_dtypes.float8_e4m3))
    Q, K = f32(inputs["Q"]), f32(inputs["K"])
    QT = np.ascontiguousarray(Q.transpose(0, 2, 1))
    KT = np.ascontiguousarray(K.transpose(0, 2, 1))
    shared = {
        "Wq": bf(inputs["Wq"]),
        "Wk": f8(inputs["Wk"]),
        "Wv": f8(inputs["Wv"]),
        "Wo": bf(inputs["Wo"]),
        "bq2": np.ascontiguousarray(f32(inputs["bq"]).reshape(DT, P).T),
        "bq": f32(inputs["bq"]),
        "bk2": np.ascontiguousarray(f32(inputs["bk"]).reshape(DT, P).T),
        "bv": f32(inputs["bv"]),
        "bo": f32(inputs["bo"]),
        "g0": f32(inputs["g0"]),
        "b0": f32(inputs["b0"]),
        "g1": f32(inputs["g1"]),
        "b1": f32(inputs["b1"]),
    }
    in_maps = []
    for c in range(NCORES):
        m = dict(shared)
        m["QT"] = np.ascontiguousarray(QT[c * BL : (c + 1) * BL].astype(ml_dtypes.bfloat16))
        m["KT"] = np.ascontiguousarray(KT[c * BL : (c + 1) * BL].astype(ml_dtypes.float8_e4m3))
        in_maps.append(m)
    return in_maps


def _run(inputs, trace=False):
    triv0 = bool(
        np.all(np.asarray(inputs["g0"]) == 1.0)
        and np.all(np.asarray(inputs["b0"]) == 0.0)
    )
    triv1 = bool(
        np.all(np.asarray(inputs["g1"]) == 1.0)
        and np.all(np.asarray(inputs["b1"]) == 0.0)
    )
    trivbo = bool(np.all(np.asarray(inputs["bo"]) == 0.0))
    trivqk = bool(
        np.all(np.asarray(inputs["bq"]) == 0.0)
        and np.all(np.asarray(inputs["bk"]) == 0.0)
    )
    nc = _get_nc(triv0, triv1, trivbo, trivqk)
    in_maps = _prep_in_maps(inputs)
    return run_bass_kernel_spmd(nc, in_maps, list(range(NCORES)), trace=trace)


def kernel(**inputs):
    res = _run(inputs, trace=False)
    return np.concatenate([res.results[c]["O"] for c in range(NCORES)], axis=0)


# revision 23
# speedup vs baseline: 1.0239x; 1.0239x over previous
"""Trainium2 Bass kernel for the MAB (multihead attention block) problem.

Full inputs in, full outputs out. Data-parallel over batch: 16 batches
across 8 NeuronCores = 2 batches/core. No collectives.

Per-core pipeline (per batch):
  1. QpT = (Q @ Wq + bq)^T bf16, KpT likewise, Vp natural fp8 (+ ones
     column per head for the softmax denominator; head stride padded to
     80 bytes for the DoubleRow LDWEIGHTS alignment rule).
  2. Per head pair: S^T = Kh @ Qh^T (row-tiled K=64 pairs sharing the PE
     array), P = exp(S^T*s) written as fp8 (no max subtraction -- scores
     are N(0, 0.35), exp is safe), O'^T[66, nq] = [Vh | 1]^T @ P
     accumulated over nk-tile PAIRS with fp8 DoubleRow matmuls (2
     contraction rows per PE cell = half the matmul count; row 64 of the
     output = softmax denominator).
  3. Transpose O'^T to natural (bf16), normalize rows by 1/denominator,
     add Qp residual.
  4. LayerNorm -> transpose -> FFN relu(X @ Wo)+X fused on DVE ->
     LayerNorm -> out. rsqrt computed as exp(-0.5*ln(var+eps)) so the
     ACT table set never leaves exp (a sqrt would force a ~2.7us
     ACT_TABLE_LOAD round-trip per switch). Affine params that are
     identically (1, 0) are folded out at build time.

Emission order is shaped for Tile's priority scheduler: a minimal
projection head (qpt/kpt chunk 0, vpa m0-1) precedes the first attention
group so the exp stream starts early; the remaining projection /
transpose chunks and the next batch's phase A are emitted between
attention groups so they fill the exp-paced PE bubbles.
"""

import math
import sys
from contextlib import ExitStack

import numpy as np

sys.path.insert(0, "/opt/trn_rl_repo")

import concourse.bass as bass
import concourse.tile as tile
from concourse import bacc
from concourse import mybir
from concourse.bass import ds, ts
from concourse.bass_utils import run_bass_kernel_spmd
from concourse.masks import make_identity

FP = mybir.dt.float32
AF = mybir.ActivationFunctionType
ALU = mybir.AluOpType
BF = mybir.dt.bfloat16
F8 = mybir.dt.float8e4
DR = mybir.MatmulPerfMode.DoubleRow

B, N, D = 16, 1024, 512
NCORES = 8
BL = B // NCORES  # batches per core
H, HD = 8, 64
SCALE = 1.0 / math.sqrt(D)
EPS = 1e-5
P = 128
DT = D // P  # 4 dv chunks
NT = N // P  # 8 nq/nk tiles
HA = HD + 1  # head dim + denominator column
HP8 = 80     # padded per-head stride in the fp8 V tile (16B-aligned)


def _bcast_ap(ap):
    """Broadcast a [D]-shaped DRAM AP across all 128 partitions."""
    return bass.AP(tensor=ap.tensor, offset=ap.offset, ap=[[0, P]] + list(ap.ap))


def _build_program(triv0, triv1, trivbo, trivqk):
    nc = bacc.Bacc(None, target_bir_lowering=False)
    dr = {}
    for name, shape in [
        ("QT", [BL, D, N]),
        ("KT", [BL, D, N]),
        ("Wq", [D, D]),
        ("Wk", [D, D]),
        ("Wv", [D, D]),
        ("Wo", [D, D]),
        ("bq2", [P, DT]),
        ("bq", [D]),
        ("bk2", [P, DT]),
        ("bv", [D]),
        ("bo", [D]),
        ("g0", [D]),
        ("b0", [D]),
        ("g1", [D]),
        ("b1", [D]),
    ]:
        if name in ("KT", "Wk", "Wv"):
            dt = F8
        elif name in ("QT", "Wq", "Wo"):
            dt = BF
        else:
            dt = FP
        dr[name] = nc.declare_dram_parameter(name, shape, dt, isOutput=False)
    out_O = nc.declare_dram_parameter("O", [BL, N, D], FP, isOutput=True)
    import os
    DBG = bool(os.environ.get("KDBG"))
    if DBG:
        dbg_qp = nc.declare_dram_parameter("DQP", [BL, NT, P, D], BF, isOutput=True)
        dbg_oasm = nc.declare_dram_parameter("DOASM", [BL, NT, P, D], BF, isOutput=True)
        dbg_ln1 = nc.declare_dram_parameter("DLN1", [BL, NT, P, D], BF, isOutput=True)

    qt_src = dr["QT"][:].rearrange("b (c p) n -> b p c n", p=P)
    kt_src = dr["KT"][:].rearrange("b (c p) n -> b p c n", p=P)

    with tile.TileContext(nc) as tc, ExitStack() as ctx:
        singles = ctx.enter_context(tc.tile_pool(name="singles", bufs=1))
        work = ctx.enter_context(tc.tile_pool(name="work", bufs=2))
        pch = ctx.enter_context(tc.tile_pool(name="pch", bufs=2))
        lnt = ctx.enter_context(tc.tile_pool(name="lnt", bufs=2))
        ost = ctx.enter_context(tc.tile_pool(name="ost", bufs=2))
        otile = ctx.enter_context(tc.tile_pool(name="otile", bufs=2))
        o2p = ctx.enter_context(tc.tile_pool(name="o2p", bufs=5))
        sml = ctx.enter_context(tc.tile_pool(name="sml", bufs=8))
        # PSUM budget: flow 2x2 banks + opv 2x1 banks + acc 2x1 banks = 8
        ps_acc = ctx.enter_context(tc.tile_pool(name="ps_acc", bufs=2, space="PSUM"))
        ps_pv = ctx.enter_context(tc.tile_pool(name="ps_pv", bufs=1, space="PSUM"))
        ps_flow = ctx.enter_context(tc.tile_pool(name="ps_flow", bufs=2, space="PSUM"))

        # ---- statics
        wsb = {}
        for wname in ("Wq", "Wk", "Wv", "Wo"):
            wdt = F8 if wname in ("Wk", "Wv") else BF
            wsb[wname] = singles.tile([P, DT, D], wdt, tag=wname, name=wname)
        nc.sync.dma_start(
            out=wsb["Wq"], in_=dr["Wq"][:].rearrange("(c p) d -> p c d", p=P)
        )
        bq_sb = singles.tile([P, DT], FP, tag="bq2")
        nc.sync.dma_start(out=bq_sb, in_=dr["bq2"][:])
        bk_sb = singles.tile([P, DT], FP, tag="bk2")
        bc = {}
        for bname in ("bq", "bv", "bo", "g0", "b0", "g1", "b1"):
            t = singles.tile([P, D], FP, tag=bname)
            nc.gpsimd.dma_start(out=t, in_=_bcast_ap(dr[bname][:]))
            bc[bname] = t
        ident = singles.tile([P, P], FP, tag="ident")
        make_identity(nc, ident)
        ident_b = singles.tile([P, P], BF, tag="identb")
        nc.vector.tensor_copy(ident_b, ident)
        eps_sb = singles.tile([P, 1], FP, tag="eps")
        nc.vector.memset(eps_sb, EPS)

        state = {}

        def rsqrt_dve(out4, var_ap):
            """out4[P,4] = 1/sqrt(var+eps) via bit-trick seed + 2 Newton
            iterations, all on DVE ALU ops (no ACT table traffic)."""
            I32 = mybir.dt.int32
            v4 = sml.tile([P, 4], FP, tag="v4", name="v4")
            nc.vector.tensor_scalar_add(v4, var_ap, EPS)
            yi = out4.bitcast(I32)
            nc.vector.tensor_scalar(
                out=yi, in0=v4.bitcast(I32), scalar1=1, scalar2=None,
                op0=ALU.arith_shift_right,
            )
            nc.vector.tensor_scalar(
                out=yi, in0=yi, scalar1=0x5F3759DF, scalar2=-1,
                op0=ALU.subtract, op1=ALU.mult,
            )
            h4 = sml.tile([P, 4], FP, tag="h4", name="h4")
            for _ in range(2):
                nc.vector.tensor_tensor(h4, out4, out4, ALU.mult)
                nc.vector.tensor_tensor(h4, h4, v4, ALU.mult)
                nc.vector.tensor_scalar(
                    out=h4, in0=h4, scalar1=-0.5, scalar2=1.5,
                    op0=ALU.mult, op1=ALU.add,
                )
                nc.vector.tensor_tensor(out4, out4, h4, ALU.mult)

        def proj_chunk(b, t):
            """One dv-chunk of the QpT (bf16) and KpT (fp8 DoubleRow)
            projections."""
            st = state[b]
            for hf in range(2):
                ps = ps_acc.tile([P, 512], FP, tag="acc", name="projps")
                for c in range(DT):
                    nc.tensor.matmul(
                        ps,
                        (wsb["Wq"][:, c, ts(t, P)]),
                        (st["qt"][:, c, ds(hf * 512, 512)]),
                        start=(c == 0),
                        stop=(c == DT - 1),
                    )
                if trivqk and b == 0:
                    nc.scalar.copy(st["qpt"][:, t, ds(hf * 512, 512)], ps)
                else:
                    nc.vector.tensor_scalar_add(
                        st["qpt"][:, t, ds(hf * 512, 512)], ps, bq_sb[:, t : t + 1]
                    )
            for hf in range(2):
                ps = ps_acc.tile([P, 512], FP, tag="acc", name="projps")
                for ci in range(DT // 2):
                    nc.tensor.matmul(
                        ps,
                        wsb["Wk"][:, ds(2 * ci, 2), ts(t, P)],
                        st["kt"][:, ds(2 * ci, 2), ds(hf * 512, 512)],
                        start=(ci == 0),
                        stop=(ci == DT // 2 - 1),
                        perf_mode=DR,
                    )
                if trivqk and b == 0:
                    nc.scalar.copy(st["kpt"][:, t, ds(hf * 512, 512)], ps)
                else:
                    nc.vector.tensor_scalar_add(
                        st["kpt"][:, t, ds(hf * 512, 512)], ps, bk_sb[:, t : t + 1]
                    )

        def vpa_chunk(b, ms):
            """Vp natural fp8 rows for nk-tiles ms."""
            st = state[b]
            kt = st["kt"]
            va = st["vpa"][:, :, :].rearrange("p m (h s) -> p m h s", s=HP8)
            for m in ms:
                ps = ps_acc.tile([P, 512], FP, tag="acc", name="vps")
                for ci in range(DT // 2):
                    nc.tensor.matmul(
                        ps,
                        kt[:, ds(2 * ci, 2), ts(m, P)],
                        wsb["Wv"][:, ds(2 * ci, 2), :],
                        start=(ci == 0),
                        stop=(ci == DT // 2 - 1),
                        perf_mode=DR,
                    )
                nc.vector.scalar_tensor_tensor(
                    out=va[:, m, :, 0:HD],
                    in0=ps[:, :].rearrange("p (h s) -> p h s", s=HD),
                    scalar=0.0,
                    in1=bc["bv"][:, :].rearrange("p (h s) -> p h s", s=HD),
                    op0=ALU.bypass,
                    op1=ALU.add,
                )

        def qp_mm(b, ms):
            """Natural-layout Qp rows (attention residual) as a direct
            matmul: qp[q, :] = Q[q, :] @ Wq + bq, with the QT input tile
            as the stationary operand. No transposes needed."""
            st = state[b]
            qt, qp = st["qt"], st["qp"]
            for m in ms:
                ps = ps_acc.tile([P, 512], FP, tag="acc", name="qpps")
                for c in range(DT):
                    nc.tensor.matmul(
                        ps,
                        (qt[:, c, ts(m, P)]),
                        (wsb["Wq"][:, c, :]),
                        start=(c == 0),
                        stop=(c == DT - 1),
                    )
                nc.vector.scalar_tensor_tensor(
                    out=qp[:, m, :],
                    in0=ps,
                    scalar=0.0,
                    in1=bc["bq"],
                    op0=ALU.bypass,
                    op1=ALU.add,
                )

        def phase_a_dma(b):
            st = {}
            state[b] = st
            qt = work.tile([P, DT, N], BF, tag="qt")
            kt = work.tile([P, DT, N], F8, tag="kt")
            for c in range(DT):
                nc.sync.dma_start(out=qt[:, c, :], in_=qt_src[b, :, c, :])
                nc.sync.dma_start(out=kt[:, c, :], in_=kt_src[b, :, c, :])
            if b == 0:
                for wname in ("Wk", "Wo", "Wv"):
                    nc.sync.dma_start(
                        out=wsb[wname],
                        in_=dr[wname][:].rearrange("(c p) d -> p c d", p=P),
                    )
                nc.sync.dma_start(out=bk_sb, in_=dr["bk2"][:])
            st["qt"], st["kt"] = qt, kt

        def phase_a_head(b):
            st = state[b]
            st["qpt"] = work.tile([P, DT, N], BF, tag="qpt", name="qpt")
            st["kpt"] = work.tile([P, DT, N], BF, tag="kpt", name="kpt")
            vpa = work.tile([P, NT, H * HP8], F8, tag="vpa", name="vpa")
            st["vpa"] = vpa
            va = vpa[:, :, :].rearrange("p m (h s) -> p m h s", s=HP8)
            nc.vector.memset(va[:, :, :, HD : HD + 1], 1.0)
            nc.vector.memset(va[:, :, :, HD + 1 : HD + 2], 0.0)
            st["qp"] = work.tile([P, NT, D], BF, tag="qp", name="qp")
            st["oasm"] = work.tile([P, NT, D], BF, tag="oasm", name="oasm")
            st["ln1"] = work.tile([P, NT, D], BF, tag="ln1", name="ln1")
            proj_chunk(b, 0)
            vpa_chunk(b, (0, 1, 2, 3, 4, 5, 6, 7))
            qp_mm(b, (0, 1, 2, 3))
            st["rest"] = [
                lambda b=b: (proj_chunk(b, 1), qp_mm(b, (4, 5))),
                lambda b=b: (proj_chunk(b, 2), qp_mm(b, (6, 7))),
                lambda b=b: (proj_chunk(b, 3),),
            ]

        def drain_group(b, hp, hf, o_pair):
            """PSUM O'^T -> natural bf16, normalize, add Qp residual."""
            st = state[b]
            qp, oasm = st["qp"], st["oasm"]
            o_sb = ost.tile([HA, N], BF, tag="ost", name="osb")
            nc.vector.tensor_copy(o_sb[:, 0:512], o_pair[0:HA, 0:512])
            nc.vector.tensor_copy(o_sb[:, 512:1024], o_pair[0:HA, 512:1024])
            for j in range(2):
                h = 2 * hp + j
                # stride 66 keeps each bf16 PSUM write 4-byte aligned
                t_ps = ps_acc.tile([P, 4 * 66], BF, tag="acc", name="otr")
                for qq in range(4):
                    nc.tensor.transpose(
                        t_ps[:, ds(qq * 66, HA)],
                        o_sb[:, ds(j * 512 + qq * P, P)],
                        ident_b[0:HA, 0:HA],
                    )
                r4 = sml.tile([P, 4], FP, tag="r", name="r4")
                den = bass.AP(
                    tensor=t_ps.tensor,
                    offset=t_ps.offset + HD,
                    ap=[list(t_ps.ap[0]), [66, 4]],
                )
                nc.vector.reciprocal(r4, den)
                for qq in range(4):
                    q = hf * 4 + qq
                    nc.vector.scalar_tensor_tensor(
                        out=oasm[:, q, ds(h * HD, HD)],
                        in0=t_ps[:, ds(qq * 66, HD)],
                        scalar=r4[:, qq : qq + 1],
                        in1=qp[:, q, ds(h * HD, HD)],
                        op0=ALU.mult,
                        op1=ALU.add,
                    )

        def attn_group(b, hf, hp, mid=None):
            st = state[b]
            qpt, kpt, vpa = st["qpt"], st["kpt"], st["vpa"]
            qslice = ds(hf * 512, 512)
            o_pair = ps_pv.tile([HA + 1, N], FP, tag="opv", name="opv")

            p8 = pch.tile([P, NT, N], F8, tag="p8")

            def emit_pv(m2):
                for j in range(2):
                    h = 2 * hp + j
                    nc.tensor.matmul(
                        o_pair[:, ds(j * 512, 512)],
                        vpa[:, ds(2 * m2, 2), ds(h * HP8, HA + 1)],
                        p8[:, ds(2 * m2, 2), ds(j * 512, 512)],
                        start=(m2 == 0),
                        stop=(m2 == NT // 2 - 1),
                        perf_mode=DR,
                    )

            pend = []
            for m in range(NT):
                s_pair = ps_flow.tile([P, N], FP, tag="flow", name="spair")
                for j in range(2):
                    lo = j * 64
                    nc.tensor.matmul(
                        s_pair[:, ds(j * 512, 512)],
                        (kpt[lo : lo + 64, hp, ts(m, P)]),
                        (qpt[lo : lo + 64, hp, qslice]),
                        start=True,
                        stop=True,
                    )
                # PV pairs and the previous group's drain are emitted after
                # this m's scores but before its exp: the scores (and thus
                # the exp stream) win scheduler priority over them.
                if m == 1 and pending_drain[0] is not None:
                    pending_drain[0]()
                    pending_drain[0] = None
                if len(pend) > 1:
                    emit_pv(pend.pop(0))
                nc.scalar.activation(p8[:, m, :], s_pair, AF.Exp, scale=SCALE)
                if mid and m in mid:
                    mid[m]()
                if m % 2 == 1:
                    pend.append(m // 2)
            for m2 in pend:
                emit_pv(m2)
            pending_drain[0] = lambda b=b, hp=hp, hf=hf, o=o_pair: drain_group(
                b, hp, hf, o
            )

        def phase_c_half(b, hf):
            """LN1 + FFN + LN2 + store for q-tiles 4*hf .. 4*hf+3."""
            st = state[b]
            oasm, ln1 = st["oasm"], st["ln1"]
            mva = sml.tile([P, 4, 2], FP, tag="mva", name="mva")
            for qq in range(4):
                st_ = sml.tile([P, 6], FP, tag="bn", name="st")
                nc.vector.bn_stats(st_, oasm[:, hf * 4 + qq, :])
                nc.vector.bn_aggr(mva[:, qq, :], st_)
            rsa = sml.tile([P, 4], FP, tag="rsa", name="rsa")
            rsqrt_dve(rsa, mva[:, :, 1])
            for qq in range(4):
                q = hf * 4 + qq
                lq = ln1[:, q, :]
                nc.vector.tensor_scalar(
                    out=lq,
                    in0=oasm[:, q, :],
                    scalar1=mva[:, qq, 0:1],
                    scalar2=rsa[:, qq : qq + 1],
                    op0=ALU.subtract,
                    op1=ALU.mult,
                )
                if not triv0:
                    nc.vector.tensor_tensor(lq, lq, bc["g0"], ALU.mult)
                    nc.vector.tensor_tensor(lq, lq, bc["b0"], ALU.add)

            mvb = sml.tile([P, 4, 2], FP, tag="mvb", name="mvb")
            o2s = []
            for qq in range(4):
                q = hf * 4 + qq
                lq = ln1[:, q, :]
                tp = ps_acc.tile([P, 512], BF, tag="acc", name="lntr")
                for c in range(DT):
                    nc.tensor.transpose(tp[:, ts(c, P)], lq[:, ts(c, P)], ident_b)
                l_t = lnt.tile([P, DT, P], BF, tag="lnt", name="lt")
                nc.vector.tensor_copy(l_t, tp.rearrange("p (c n) -> p c n", n=P))

                f_ps = ps_acc.tile([P, 512], FP, tag="acc", name="ffps")
                for c in range(DT):
                    nc.tensor.matmul(
                        f_ps,
                        (l_t[:, c, :]),
                        (wsb["Wo"][:, c, :]),
                        start=(c == 0),
                        stop=(c == DT - 1),
                    )
                o2 = o2p.tile([P, D], BF, tag="o2", name="o2")
                o2s.append(o2)
                if trivbo:
                    # o2 = relu(f_ps) + ln1 in one DVE op
                    nc.vector.scalar_tensor_tensor(
                        out=o2,
                        in0=f_ps,
                        scalar=0.0,
                        in1=lq,
                        op0=ALU.max,
                        op1=ALU.add,
                    )
                else:
                    nc.vector.tensor_tensor(o2, f_ps, bc["bo"], ALU.add)
                    nc.vector.tensor_scalar_max(o2, o2, 0.0)
                    nc.vector.tensor_tensor(o2, o2, lq, ALU.add)
                st2 = sml.tile([P, 6], FP, tag="bn", name="st2")
                nc.vector.bn_stats(st2, o2)
                nc.vector.bn_aggr(mvb[:, qq, :], st2)
            rsb = sml.tile([P, 4], FP, tag="rsb", name="rsb")
            rsqrt_dve(rsb, mvb[:, :, 1])
            for qq in range(4):
                q = hf * 4 + qq
                z2 = otile.tile([P, D], FP, tag="z", name="z2")
                nc.vector.tensor_scalar(
                    out=z2,
                    in0=o2s[qq],
                    scalar1=mvb[:, qq, 0:1],
                    scalar2=rsb[:, qq : qq + 1],
                    op0=ALU.subtract,
                    op1=ALU.mult,
                )
                if not triv1:
                    nc.vector.tensor_tensor(z2, z2, bc["g1"], ALU.mult)
                    nc.vector.tensor_tensor(z2, z2, bc["b1"], ALU.add)
                nc.sync.dma_start(out=out_O[b, ts(q, P), :], in_=z2)

        # ---- emission order (shapes scheduler priorities) ----
        pending_drain = [None]

        def flush_drain():
            if pending_drain[0] is not None:
                pending_drain[0]()
                pending_drain[0] = None

        phase_a_dma(0)
        phase_a_head(0)
        while state[0]["rest"]:
            state[0]["rest"].pop(0)()
        for b in range(BL):
            for hp in range(4):
                attn_group(b, 0, hp)
            if b + 1 < BL:
                # input DMAs submitted early: the sync DGE queue is FIFO,
                # so they must precede the c-half work on that queue
                phase_a_dma(b + 1)
            if b + 1 == BL:
                # last batch: overlap the first c-half with hf=1 attention
                flush_drain()
                phase_c_half(b, 0)
            for hp in range(4):
                attn_group(b, 1, hp)
            flush_drain()
            # emitted after all of this batch's groups: lower priority,
            # so they fill the exp-paced PE bubbles instead of delaying
            # groups. Next batch's projections go FIRST so its attention
            # restarts promptly; this batch's c-halves then overlap it.
            if b + 1 < BL:
                phase_a_head(b + 1)
                while state[b + 1]["rest"]:
                    state[b + 1]["rest"].pop(0)()
                phase_c_half(b, 0)
            phase_c_half(b, 1)
            if DBG:
                for m in range(NT):
                    nc.sync.dma_start(out=dbg_qp[b, m], in_=state[b]["qp"][:, m, :])
                    nc.sync.dma_start(out=dbg_oasm[b, m], in_=state[b]["oasm"][:, m, :])
                    nc.sync.dma_start(out=dbg_ln1[b, m], in_=state[b]["ln1"][:, m, :])

    nc.compile()
    return nc


_NC = {}


def _get_nc(triv0, triv1, trivbo, trivqk=True):
    key = (triv0, triv1, trivbo, trivqk)
    if key not in _NC:
        _NC[key] = _build_program(*key)
    return _NC[key]


def _prep_in_maps(inputs):
    import ml_dtypes

    f32 = lambda x: np.ascontiguousarray(np.asarray(x), dtype=np.float32)
    bf = lambda x: np.ascontiguousarray(np.asarray(x, dtype=np.float32).astype(ml_dtypes.bfloat16))
    f8 = lambda x: np.ascontiguousarray(np.asarray(x, dtype=np.float32).astype(ml_dtypes.float8_e4m3))
    Q, K = f32(inputs["Q"]), f32(inputs["K"])
    QT = np.ascontiguousarray(Q.transpose(0, 2, 1))
    KT = np.ascontiguousarray(K.transpose(0, 2, 1))
    shared = {
        "Wq": bf(inputs["Wq"]),
        "Wk": f8(inputs["Wk"]),
        "Wv": f8(inputs["Wv"]),
        "Wo": bf(inputs["Wo"]),
        "bq2": np.ascontiguousarray(f32(inputs["bq"]).reshape(DT, P).T),
        "bq": f32(inputs["bq"]),
        "bk2": np.ascontiguousarray(f32(inputs["bk"]).reshape(DT, P).T),
        "bv": f32(inputs["bv"]),
        "bo": f32(inputs["bo"]),
        "g0": f32(inputs["g0"]),
        "b0": f32(inputs["b0"]),
        "g1": f32(inputs["g1"]),
        "b1": f32(inputs["b1"]),
    }
    in_maps = []
    for c in range(NCORES):
        m = dict(shared)
        m["QT"] = np.ascontiguousarray(QT[c * BL : (c + 1) * BL].astype(ml_dtypes.bfloat16))
        m["KT"] = np.ascontiguousarray(KT[c * BL : (c + 1) * BL].astype(ml_dtypes.float8_e4m3))
        in_maps.append(m)
    return in_maps


def _run(inputs, trace=False):
    triv0 = bool(
        np.all(np.asarray(inputs["g0"]) == 1.0)
        and np.all(np.asarray(inputs["b0"]) == 0.0)
    )
    triv1 = bool(
        np.all(np.asarray(inputs["g1"]) == 1.0)
        and np.all(np.asarray(inputs["b1"]) == 0.0)
    )
    trivbo = bool(np.all(np.asarray(inputs["bo"]) == 0.0))
    trivqk = bool(
        np.all(np.asarray(inputs["bq"]) == 0.0)
        and np.all(np.asarray(inputs["bk"]) == 0.0)
    )
    nc = _get_nc(triv0, triv1, trivbo, trivqk)
    in_maps = _prep_in_maps(inputs)
    return run_bass_kernel_spmd(nc, in_maps, list(range(NCORES)), trace=trace)


def kernel(**inputs):
    res = _run(inputs, trace=False)
    return np.concatenate([res.results[c]["O"] for c in range(NCORES)], axis=0)
